# revision 34
# baseline (speedup 1.0000x reference)
"""Bass/Tile kernel for nn_MultiHeadAttention (B=2, S=2048, D=1024, H=16) on 8 trn2 cores.

Sharding: core c -> (b = c//4, head-group hg = c%4). Each core computes 4 heads'
q/k/v projections, relu-attention, and a partial FC (256 of 1024 contraction rows).
Host pre-casts to bf16, pre-transposes x / weight slices, and sums the 4
partials per batch + bias.

v10 design (baseline v3 structure + absorbed projections):
  - attention windows keep the v3 per-m interleave (scores pair, av pair lag 2,
    one filler slot every other m) - measured fastest: window pace is set by
    the PSUM drain (two single-bank relus per m, one on DVE one on ACT,
    ~680ns/m) and the fine interleave lets the Tile scheduler fill PE gaps
  - scores pairs 2x row-tiled (tile_position (0,0)/(64,0)); av pairs 2x
    col-tiled ((0,0)/(0,64)) - concurrent PE tiles
  - v-projection and the hp1 half of the k-projection are injected into the
    qb0 windows' drain slack; q-projections for qb+1 and fc for qb-1 are
    injected into later windows (v3 ran k/v as standalone PE phases with
    DVE/ACT idle)
  - y output bf16 (halves the 8MB output DMA), staged via [P,1024] tiles
  - weights/x ride the sync+scalar HWDGE queues (gpsimd DMA = slow SWDGE),
    wv/wfc needed late ride gpsimd
"""
import numpy as np
import ml_dtypes

import concourse.bass as bass
import concourse.mybir as mybir
import concourse.tile as tile

F32 = mybir.dt.float32
BF16 = mybir.dt.bfloat16
ts, ds = bass.ts, bass.ds

S = 2048
D = 1024
DL = 256      # per-core q/k/v dim (4 heads x 64)
P = 128
KD = D // P   # 8 k-chunks for projections
SQ = 512      # q-block (matmul N)
NQB = S // SQ # 4
NM = S // P   # 16 kpos chunks
DLC = DL // P # 2


def split_excess_waits(nc, max_embed: int = 1):
    """walrus core_v3 codegen accepts at most one sync-wait per instruction;
    move extra waits onto standalone event-sem instructions inserted before."""
    n_split = 0
    counter = 0
    for f in nc.m.functions:
        for blk in f.blocks:
            insts = blk.instructions
            if not any(
                ins.sync_info is not None and len(ins.sync_info.on_wait) > max_embed
                for ins in insts
            ):
                continue
            newl = []
            for ins in insts:
                si = ins.sync_info
                if si is not None and len(si.on_wait) > max_embed:
                    waits = list(si.on_wait)
                    extra, keep = waits[:-max_embed], waits[-max_embed:]
                    for w in extra:
                        counter += 1
                        es = mybir.InstEventSemaphore(name=f"waitsplit_{counter}")
                        es.engine = ins.engine
                        es.sync_info = mybir.SyncInfo(on_wait=[w], on_update=[])
                        newl.append(es)
                        n_split += 1
                    si.on_wait = keep
                newl.append(ins)
            blk.instructions = newl
    return n_split


def build_nc(with_mask: bool):
    nc = bass.Bass()
    # pre-arranged on host: x[p, c, s] = x.T[128c+p, s]; w[p, c, f] = w.T[128c+p, f]
    xT = nc.dram_tensor("xT", [P, KD, S], BF16, kind="ExternalInput")
    wq = nc.dram_tensor("wq", [P, KD, DL], BF16, kind="ExternalInput")
    wk = nc.dram_tensor("wk", [P, KD, DL], BF16, kind="ExternalInput")
    wv = nc.dram_tensor("wv", [P, KD, DL], BF16, kind="ExternalInput")
    wfc = nc.dram_tensor("wfc", [P, DLC, D], BF16, kind="ExternalInput")
    maskT = nc.dram_tensor("maskT", [S, S], F32, kind="ExternalInput") if with_mask else None
    y = nc.dram_tensor("y", [S, D], BF16, kind="ExternalOutput")

    with tile.TileContext(nc) as tc:
        _Emitter(tc, xT, wq, wk, wv, wfc, maskT, y).run()
    split_excess_waits(nc)
    return nc


class _Emitter:
    def __init__(self, tc, xT, wq, wk, wv, wfc, maskT, y):
        self.tc = tc
        self.nc = tc.nc
        self.xT, self.wq, self.wk, self.wv, self.wfc = xT, wq, wk, wv, wfc
        self.maskT, self.y = maskT, y
        self.cp = 0
        self.rl = 0

    # -- engine alternation helpers ----------------------------------------
    def copyback(self, out_ap, in_ap):
        if self.cp % 2 == 0:
            self.nc.vector.tensor_copy(out_ap, in_ap)
        else:
            self.nc.scalar.copy(out_ap, in_ap)
        self.cp += 1

    def relu(self, out_ap, in_ap):
        if self.rl % 2 == 0:
            self.nc.vector.tensor_scalar_max(out_ap, in_ap, 0.0)
        else:
            self.nc.scalar.activation(out_ap, in_ap, mybir.ActivationFunctionType.Relu)
        self.rl += 1

    # -- emission pieces ----------------------------------------------------
    def kq_group(self, wsb, dstT, c, nb):
        """one projection psum group: dstT[:, c, nb*SQ:...] via 8 k-chunk matmuls"""
        nc = self.nc
        pt = self.ps.tile([P, SQ], F32, tag="fl", bufs=1, name=f"pj_{dstT.name}_{c}_{nb}")
        for k in range(KD):
            nc.tensor.matmul(
                pt[:], wsb[:, k, ts(c, P)], self.xb[:, k, ds(nb * SQ, SQ)],
                start=(k == 0), stop=(k == KD - 1),
            )
        self.copyback(dstT[:, c, ds(nb * SQ, SQ)], pt[:])

    def v_group(self, m):
        nc = self.nc
        pt = self.ps.tile([P, DL], F32, tag="fl", bufs=1, name=f"v_{m}", padded_shape=[P, SQ])
        for k in range(KD):
            nc.tensor.matmul(
                pt[:], self.xb[:, k, ts(m, P)], self.wv_sb[:, k, :],
                start=(k == 0), stop=(k == KD - 1),
            )
        self.copyback(self.vN[:, m, :], pt[:])

    def scores_single(self, qb, hp, m, attn_t, mtile):
        """one scores pair: 2 row-tiled concurrent matmuls into a 2-bank tile,
        drained by ONE paired relu (engines alternate per pair, so in a 2-pair
        batch both engines drain concurrently: ~20% less engine time than
        single-bank relus and same latency)"""
        nc = self.nc
        pt = self.ps.tile([P, 2, SQ], F32, tag="sc", bufs=2, name=f"sc_{qb}_{hp}_{m}")
        for h in range(2):
            nc.tensor.matmul(
                pt[:, h, :],
                self.kT[ds(64 * h, 64), hp, ts(m, P)],
                self.qT[ds(64 * h, 64), hp, ds(qb * SQ, SQ)],
                start=True, stop=True,
                tile_position=(64 * h, 0),
            )
        if mtile is not None:
            for h in range(2):
                nc.vector.tensor_tensor(
                    pt[:, h, :], pt[:, h, :], mtile[:, m, :], mybir.AluOpType.add
                )
        self.relu(attn_t[:, m, :, :], pt[:, :, :])

    def av(self, qb, hp, m, attn_t, po):
        nc = self.nc
        for h in range(2):
            nc.tensor.matmul(
                po[ds(64 * h, 64), :],
                self.vN[:, m, ds(128 * hp + 64 * h, 64)],
                attn_t[:, m, h, :],
                start=(m == 0), stop=(m == NM - 1),
                tile_position=(0, 64 * h),
            )

    def fc_group(self, sc, eb):
        """single-bank fc group: y rows [sc] cols [eb-half]; staged to bf16"""
        nc = self.nc
        pt = self.ps_fc.tile([P, SQ], F32, tag="fc", name=f"fc_{sc}_{eb}")
        for c in range(DLC):
            nc.tensor.matmul(
                pt[:], self.outT[:, c, ts(sc, P)], self.wfc_sb[:, c, ds(eb * SQ, SQ)],
                start=(c == 0), stop=(c == DLC - 1),
            )
        if eb == 0:
            self.cur_yt = self.ystage.tile([P, D], BF16, tag="yt", name=f"yt_{sc}")
        self.copyback(self.cur_yt[:, ds(eb * SQ, SQ)], pt[:])
        if eb == 1:
            nc.sync.dma_start(self.y[ts(sc, P), :], self.cur_yt[:])

    def inject(self, n=1):
        for _ in range(n):
            if self.pending:
                self.pending.pop(0)()

    def load_mask(self, qb):
        if self.maskT is None:
            return None
        nc = self.nc
        mtile = self.mstg.tile([P, NM, SQ], F32, tag="mask", name=f"mask_{qb}")
        for m in range(NM):
            nc.gpsimd.dma_start(
                mtile[:, m, :],
                self.maskT[:, :].rearrange("(m p) q -> p m q", p=P)[:, m, ds(qb * SQ, SQ)],
            )
        return mtile

    def window(self, qb, hp, mt, inj=None, batch2=True):
        """attention window: scores pairs, av pairs (lag 4), injections.
        inj: dict m -> list of group fns (k/v/q/fc) to emit at that iteration;
        self.pending drains one slot every other m when no explicit inj."""
        at = self.attn_pool.tile([P, NM, 2, SQ], BF16, tag="attn", name=f"attn_{qb}_{hp}")
        po = self.ps_av.tile([P, SQ], F32, tag="av", bufs=1, name=f"av_{qb}_{hp}")
        inj = inj or {}
        if batch2:
            # 2-pair scores / 2-pair av (lag 4) batches: halves the row<->col
            # mode transitions vs per-m alternation (~110ns per transition)
            for m in range(NM + 4):
                for f in inj.get(m, ()):
                    f()
                if m < NM and m % 2 == 0:
                    self.scores_single(qb, hp, m, at, mt)
                    self.scores_single(qb, hp, m + 1, at, mt)
                if m >= 4 and m % 2 == 0:
                    self.av(qb, hp, m - 4, at, po)
                    self.av(qb, hp, m - 3, at, po)
                if m % 2 == 1 and m not in inj:
                    self.inject(1)
        else:
            # per-m interleave with av lag 4: used for the v-heavy qb0 hp0
            # window where an injected group sits between v(m) and av(m)
            for m in range(NM + 4):
                for f in inj.get(m, ()):
                    f()
                if m < NM:
                    self.scores_single(qb, hp, m, at, mt)
                if m >= 4:
                    self.av(qb, hp, m - 4, at, po)
        self.copyback(self.outT[:, hp, ds(qb * SQ, SQ)], po[:])

    # -- main ---------------------------------------------------------------
    def run(self):
        from contextlib import ExitStack

        tc, nc = self.tc, self.nc
        stack = ExitStack()
        sb = stack.enter_context(tc.tile_pool(name="sb", bufs=1))
        # PSUM budget (8 banks): sc pairs 2x2, filler 1, fc 2, av 1
        self.ps = stack.enter_context(tc.tile_pool(name="ps", bufs=4, space="PSUM"))
        self.ps_fc = stack.enter_context(tc.tile_pool(name="ps_fc", bufs=2, space="PSUM"))
        self.ps_av = stack.enter_context(tc.tile_pool(name="ps_av", bufs=2, space="PSUM"))
        self.attn_pool = stack.enter_context(tc.tile_pool(name="attn", bufs=2))
        self.mstg = stack.enter_context(tc.tile_pool(name="mstg", bufs=2))
        self.ystage = stack.enter_context(tc.tile_pool(name="ystage", bufs=2))

        self.xb = sb.tile([P, KD, S], BF16, name="xb")
        self.wq_sb = sb.tile([P, KD, DL], BF16, name="wq_sb")
        self.wk_sb = sb.tile([P, KD, DL], BF16, name="wk_sb")
        self.wv_sb = sb.tile([P, KD, DL], BF16, name="wv_sb")
        self.wfc_sb = sb.tile([P, DLC, D], BF16, name="wfc_sb")
        self.qT = sb.tile([P, DLC, S], BF16, name="qT")
        self.kT = sb.tile([P, DLC, S], BF16, name="kT")
        self.vN = sb.tile([P, NM, DL], BF16, name="vN")
        self.outT = sb.tile([P, DLC, S], BF16, name="outT")
        self.pending = []

        # loads on the sync+scalar HWDGE queues; gpsimd (slow SWDGE) only for
        # wv/wfc which are needed late. wq + x block0 first for the q0 start.
        nc.scalar.dma_start(self.wq_sb[:], self.wq[:, :, :])
        for k in range(4):
            nc.sync.dma_start(self.xb[:, k, ds(0, SQ)], self.xT[:, k, ds(0, SQ)])
            nc.scalar.dma_start(self.xb[:, k + 4, ds(0, SQ)], self.xT[:, k + 4, ds(0, SQ)])
        nc.scalar.dma_start(self.wk_sb[:], self.wk[:, :, :])
        for k in range(4):
            nc.sync.dma_start(self.xb[:, k, ds(SQ, SQ)], self.xT[:, k, ds(SQ, SQ)])
            nc.scalar.dma_start(self.xb[:, k + 4, ds(SQ, SQ)], self.xT[:, k + 4, ds(SQ, SQ)])
        # wv must beat the v-group injections in window (0,0): HWDGE, not the
        # slow gpsimd SWDGE path (a late wv stalls the whole compile-time-
        # ordered PE queue behind the first v group)
        nc.sync.dma_start(self.wv_sb[:], self.wv[:, :, :])
        for k in range(4):
            nc.sync.dma_start(self.xb[:, k, ds(S // 2, S // 2)], self.xT[:, k, ds(S // 2, S // 2)])
            nc.scalar.dma_start(self.xb[:, k + 4, ds(S // 2, S // 2)], self.xT[:, k + 4, ds(S // 2, S // 2)])
        nc.gpsimd.dma_start(self.wfc_sb[:], self.wfc[:, :, :])

        K = lambda c, nb: self.kq_group(self.wk_sb, self.kT, c, nb)
        Q = lambda c, nb: self.kq_group(self.wq_sb, self.qT, c, nb)

        def fK(c, nb):
            return lambda: K(c, nb)
        def fQ(c, nb):
            return lambda: Q(c, nb)
        def fV(m):
            return lambda: self.v_group(m)
        def fFC(sc, eb):
            return lambda: self.fc_group(sc, eb)

        # upfront: q block 0 + first half of k(c0) - only what scores(0,0)
        # m<8 needs, so the window starts as soon as x blocks 0-1 land
        mt0 = self.load_mask(0)
        Q(0, 0)
        Q(1, 0)
        K(0, 0)
        K(0, 1)

        # qb0 hp0: inject the v-projection (av lag 4 leaves v(m) two its of
        # lead time), the rest of k(c0), and the first k(c1) groups
        inj00 = {j: [fV(j)] for j in range(NM)}
        inj00[6].insert(0, fK(0, 2))
        inj00[10].insert(0, fK(0, 3))
        inj00[16] = [fK(1, 0)]
        inj00[17] = [fK(1, 1)]
        self.window(0, 0, mt0, inj=inj00, batch2=False)
        # qb0 hp1: remaining k(c1) + q block 1
        self.window(0, 1, mt0, inj={
            1: [fK(1, 2)], 5: [fK(1, 3)], 9: [fQ(0, 1)], 13: [fQ(1, 1)],
        })

        for qb in range(1, NQB):
            mt = self.load_mask(qb)
            for hp in range(DLC):
                a = (qb - 1) * 4 + 2 * hp  # fc seq chunks of the previous qb
                self.pending += [fFC(a, 0), fFC(a, 1), fFC(a + 1, 0), fFC(a + 1, 1)]
                if qb < NQB - 1:
                    self.pending.append(fQ(hp, qb + 1))
                self.window(qb, hp, mt)

        # tail: fc for the last qb
        for sc in range(12, 16):
            for eb in range(2):
                self.fc_group(sc, eb)
        self.inject(len(self.pending))

        stack.close()


# ---- host wrapper ---------------------------------------------------------

N_HEAD = 16
_nc_cache = {}


def get_nc(with_mask: bool):
    if with_mask not in _nc_cache:
        _nc_cache[with_mask] = build_nc(with_mask)
    return _nc_cache[with_mask]


def make_in_maps(x, mask, Wq, Wk, Wv, Wfc, with_mask):
    scale = np.float32(1.0 / np.sqrt(D // N_HEAD))
    bf = ml_dtypes.bfloat16
    in_maps = []
    for c in range(8):
        b, hg = divmod(c, 4)
        gs = slice(DL * hg, DL * hg + DL)
        def prearrange(wT, cdim):  # [cdim*128, F] -> [128, cdim, F]
            F = wT.shape[1]
            return np.ascontiguousarray(
                wT.reshape(cdim, P, F).transpose(1, 0, 2)
            ).astype(bf)

        m = {
            "xT": prearrange(x[b].T, KD),
            "wq": prearrange((Wq[gs, :] * scale).T, KD),
            "wk": prearrange(Wk[gs, :].T, KD),
            "wv": prearrange(Wv[gs, :].T, KD),
            "wfc": prearrange(Wfc[:, gs].T, DLC),
        }
        if with_mask:
            m["maskT"] = np.ascontiguousarray(
                np.broadcast_to(mask, (1, 1, S, S))[0, 0].T.astype(np.float32)
            )
        in_maps.append(m)
    return in_maps


def kernel(x, mask, Wq, Wk, Wv, Wfc, bfc):
    """Full-input entry: shards across 8 trn2 cores, returns the full output."""
    from concourse.bass_utils import run_bass_kernel_spmd

    x = np.asarray(x, dtype=np.float32)
    mask = np.asarray(mask, dtype=np.float32)
    Wq = np.asarray(Wq, dtype=np.float32)
    Wk = np.asarray(Wk, dtype=np.float32)
    Wv = np.asarray(Wv, dtype=np.float32)
    Wfc = np.asarray(Wfc, dtype=np.float32)
    bfc = np.asarray(bfc, dtype=np.float32)

    B = x.shape[0]
    with_mask = bool(np.any(mask))
    nc = get_nc(with_mask)
    in_maps = make_in_maps(x, mask, Wq, Wk, Wv, Wfc, with_mask)

    res = run_bass_kernel_spmd(nc, in_maps, core_ids=list(range(8)))
    parts = np.stack([np.asarray(r["y"]) for r in res.results])  # [8, S, D] bf16
    out = parts.astype(np.float64).reshape(B, 4, S, D).sum(axis=1)
    out += bfc.astype(np.float64)
    return out.astype(np.float32)


# revision 38
# speedup vs baseline: 1.0889x; 1.0889x over previous
"""Bass/Tile kernel for nn_MultiHeadAttention (B=2, S=2048, D=1024, H=16) on 8 trn2 cores.

Sharding: core c -> (b = c//4, head-group hg = c%4). Each core computes 4 heads'
q/k/v projections, relu-attention, and a partial FC (256 of 1024 contraction rows).
Host pre-casts to bf16, pre-transposes x / weight slices, and sums the 4
partials per batch + bias.

v10 design (baseline v3 structure + absorbed projections):
  - attention windows keep the v3 per-m interleave (scores pair, av pair lag 2,
    one filler slot every other m) - measured fastest: window pace is set by
    the PSUM drain (two single-bank relus per m, one on DVE one on ACT,
    ~680ns/m) and the fine interleave lets the Tile scheduler fill PE gaps
  - scores pairs 2x row-tiled (tile_position (0,0)/(64,0)); av pairs 2x
    col-tiled ((0,0)/(0,64)) - concurrent PE tiles
  - v-projection and the hp1 half of the k-projection are injected into the
    qb0 windows' drain slack; q-projections for qb+1 and fc for qb-1 are
    injected into later windows (v3 ran k/v as standalone PE phases with
    DVE/ACT idle)
  - y output bf16 (halves the 8MB output DMA), staged via [P,1024] tiles
  - weights/x ride the sync+scalar HWDGE queues (gpsimd DMA = slow SWDGE),
    wv/wfc needed late ride gpsimd
"""
import numpy as np
import ml_dtypes

import concourse.bass as bass
import concourse.mybir as mybir
import concourse.tile as tile

F32 = mybir.dt.float32
BF16 = mybir.dt.bfloat16
ts, ds = bass.ts, bass.ds

S = 2048
D = 1024
DL = 256      # per-core q/k/v dim (4 heads x 64)
P = 128
KD = D // P   # 8 k-chunks for projections
SQ = 512      # q-block (matmul N)
NQB = S // SQ # 4
NM = S // P   # 16 kpos chunks
DLC = DL // P # 2


def split_excess_waits(nc, max_embed: int = 1):
    """walrus core_v3 codegen accepts at most one sync-wait per instruction;
    move extra waits onto standalone event-sem instructions inserted before."""
    n_split = 0
    counter = 0
    for f in nc.m.functions:
        for blk in f.blocks:
            insts = blk.instructions
            if not any(
                ins.sync_info is not None and len(ins.sync_info.on_wait) > max_embed
                for ins in insts
            ):
                continue
            newl = []
            for ins in insts:
                si = ins.sync_info
                if si is not None and len(si.on_wait) > max_embed:
                    waits = list(si.on_wait)
                    extra, keep = waits[:-max_embed], waits[-max_embed:]
                    for w in extra:
                        counter += 1
                        es = mybir.InstEventSemaphore(name=f"waitsplit_{counter}")
                        es.engine = ins.engine
                        es.sync_info = mybir.SyncInfo(on_wait=[w], on_update=[])
                        newl.append(es)
                        n_split += 1
                    si.on_wait = keep
                newl.append(ins)
            blk.instructions = newl
    return n_split


def build_nc(with_mask: bool):
    nc = bass.Bass()
    # pre-arranged on host: x[p, c, s] = x.T[128c+p, s]; w[p, c, f] = w.T[128c+p, f]
    xT = nc.dram_tensor("xT", [P, KD, S], BF16, kind="ExternalInput")
    wq = nc.dram_tensor("wq", [P, KD, DL], BF16, kind="ExternalInput")
    wk = nc.dram_tensor("wk", [P, KD, DL], BF16, kind="ExternalInput")
    wv = nc.dram_tensor("wv", [P, KD, DL], BF16, kind="ExternalInput")
    wfc = nc.dram_tensor("wfc", [P, DLC, D], BF16, kind="ExternalInput")
    maskT = nc.dram_tensor("maskT", [S, S], F32, kind="ExternalInput") if with_mask else None
    y = nc.dram_tensor("y", [S, D], BF16, kind="ExternalOutput")

    with tile.TileContext(nc) as tc:
        _Emitter(tc, xT, wq, wk, wv, wfc, maskT, y).run()
    split_excess_waits(nc)
    return nc


class _Emitter:
    def __init__(self, tc, xT, wq, wk, wv, wfc, maskT, y):
        self.tc = tc
        self.nc = tc.nc
        self.xT, self.wq, self.wk, self.wv, self.wfc = xT, wq, wk, wv, wfc
        self.maskT, self.y = maskT, y
        self.cp = 0
        self.rl = 0

    # -- engine alternation helpers ----------------------------------------
    def copyback(self, out_ap, in_ap):
        if self.cp % 2 == 0:
            self.nc.vector.tensor_copy(out_ap, in_ap)
        else:
            self.nc.scalar.copy(out_ap, in_ap)
        self.cp += 1

    def relu(self, out_ap, in_ap):
        if self.rl % 2 == 0:
            self.nc.vector.tensor_scalar_max(out_ap, in_ap, 0.0)
        else:
            self.nc.scalar.activation(out_ap, in_ap, mybir.ActivationFunctionType.Relu)
        self.rl += 1

    # -- emission pieces ----------------------------------------------------
    def kq_group(self, wsb, dstT, c, nb):
        """one projection psum group: dstT[:, c, nb*SQ:...] via 8 k-chunk matmuls"""
        nc = self.nc
        pt = self.ps.tile([P, SQ], F32, tag="pp", name=f"pj_{dstT.name}_{c}_{nb}")
        for k in range(KD):
            nc.tensor.matmul(
                pt[:], wsb[:, k, ts(c, P)], self.xb[:, k, ds(nb * SQ, SQ)],
                start=(k == 0), stop=(k == KD - 1),
            )
        self.copyback(dstT[:, c, ds(nb * SQ, SQ)], pt[:])

    def v_group(self, m):
        nc = self.nc
        pt = self.ps.tile([P, DL], F32, tag="pp", name=f"v_{m}")
        for k in range(KD):
            nc.tensor.matmul(
                pt[:], self.xb[:, k, ts(m, P)], self.wv_sb[:, k, :],
                start=(k == 0), stop=(k == KD - 1),
            )
        self.copyback(self.vN[:, m, :], pt[:])

    def scores_single(self, qb, hp, m, attn_t, mtile):
        """one scores pair: 2 row-tiled concurrent matmuls, two single-bank
        relus alternating DVE/ACT (measured best: paired [P,2,SQ] relus double
        the drain latency per pair and stall the 2-pair PSUM rotation)"""
        nc = self.nc
        pts = []
        for h in range(2):
            pt = self.ps.tile([P, SQ], F32, tag="pp", name=f"sc_{qb}_{hp}_{m}_{h}")
            nc.tensor.matmul(
                pt[:],
                self.kT[ds(64 * h, 64), hp, ts(m, P)],
                self.qT[ds(64 * h, 64), hp, ds(qb * SQ, SQ)],
                start=True, stop=True,
                tile_position=(64 * h, 0),
            )
            pts.append(pt)
        for h in range(2):
            pt = pts[h]
            if mtile is not None:
                nc.vector.tensor_tensor(
                    pt[:], pt[:], mtile[:, m, :], mybir.AluOpType.add
                )
            self.relu(attn_t[:, m, h, :], pt[:])

    def av(self, qb, hp, m, attn_t, po):
        nc = self.nc
        for h in range(2):
            nc.tensor.matmul(
                po[ds(64 * h, 64), :],
                self.vN[:, m, ds(128 * hp + 64 * h, 64)],
                attn_t[:, m, h, :],
                start=(m == 0), stop=(m == NM - 1),
                tile_position=(0, 64 * h),
            )

    def fc_group(self, sc, eb, tail=False):
        """single-bank fc group: y rows [sc] cols [eb-half]; staged to bf16.
        tail groups alternate into the pp pool (free after the last scores)
        so the final 8 groups don't serialize on 2 fc banks."""
        nc = self.nc
        if tail and (sc + eb) % 2 == 0:
            pt = self.ps.tile([P, SQ], F32, tag="pp", name=f"fc_{sc}_{eb}")
        else:
            pt = self.ps_fc.tile([P, SQ], F32, tag="fc", name=f"fc_{sc}_{eb}")
        for c in range(DLC):
            nc.tensor.matmul(
                pt[:], self.outT[:, c, ts(sc, P)], self.wfc_sb[:, c, ds(eb * SQ, SQ)],
                start=(c == 0), stop=(c == DLC - 1),
            )
        if eb == 0:
            self.cur_yt = self.ystage.tile([P, D], BF16, tag="yt", name=f"yt_{sc}")
        self.copyback(self.cur_yt[:, ds(eb * SQ, SQ)], pt[:])
        if eb == 1:
            nc.sync.dma_start(self.y[ts(sc, P), :], self.cur_yt[:])

    def inject(self, n=1):
        for _ in range(n):
            if self.pending:
                self.pending.pop(0)()

    def load_mask(self, qb):
        if self.maskT is None:
            return None
        nc = self.nc
        mtile = self.mstg.tile([P, NM, SQ], F32, tag="mask", name=f"mask_{qb}")
        for m in range(NM):
            nc.gpsimd.dma_start(
                mtile[:, m, :],
                self.maskT[:, :].rearrange("(m p) q -> p m q", p=P)[:, m, ds(qb * SQ, SQ)],
            )
        return mtile

    def window(self, qb, hp, mt, inj=None, batch2=True):
        """attention window: scores pairs, av pairs (lag 4), injections.
        inj: dict m -> list of group fns (k/v/q/fc) to emit at that iteration;
        self.pending drains one slot every other m when no explicit inj."""
        at = self.attn_pool.tile([P, NM, 2, SQ], BF16, tag="attn", name=f"attn_{qb}_{hp}")
        po = self.ps_av.tile([P, SQ], F32, tag="av", name=f"av_{qb}_{hp}")
        inj = inj or {}
        if batch2:
            # 2-pair scores / 2-pair av (lag 4) batches: halves the row<->col
            # mode transitions vs per-m alternation (~110ns per transition)
            for m in range(NM + 4):
                for f in inj.get(m, ()):
                    f()
                if m < NM and m % 2 == 0:
                    self.scores_single(qb, hp, m, at, mt)
                    self.scores_single(qb, hp, m + 1, at, mt)
                if m >= 4 and m % 2 == 0:
                    self.av(qb, hp, m - 4, at, po)
                    self.av(qb, hp, m - 3, at, po)
                if m % 2 == 1 and m not in inj:
                    self.inject(1)
        else:
            # per-m interleave with av lag 4: used for the v-heavy qb0 hp0
            # window where an injected group sits between v(m) and av(m)
            for m in range(NM + 4):
                for f in inj.get(m, ()):
                    f()
                if m < NM:
                    self.scores_single(qb, hp, m, at, mt)
                if m >= 4:
                    self.av(qb, hp, m - 4, at, po)
        self.copyback(self.outT[:, hp, ds(qb * SQ, SQ)], po[:])

    # -- main ---------------------------------------------------------------
    def run(self):
        from contextlib import ExitStack

        tc, nc = self.tc, self.nc
        stack = ExitStack()
        sb = stack.enter_context(tc.tile_pool(name="sb", bufs=1))
        # PSUM budget (8 banks): pp 4, fc 2, av 2
        self.ps = stack.enter_context(tc.tile_pool(name="ps", bufs=4, space="PSUM"))
        self.ps_fc = stack.enter_context(tc.tile_pool(name="ps_fc", bufs=2, space="PSUM"))
        self.ps_av = stack.enter_context(tc.tile_pool(name="ps_av", bufs=2, space="PSUM"))
        self.attn_pool = stack.enter_context(tc.tile_pool(name="attn", bufs=2))
        self.mstg = stack.enter_context(tc.tile_pool(name="mstg", bufs=2))
        self.ystage = stack.enter_context(tc.tile_pool(name="ystage", bufs=4))

        self.xb = sb.tile([P, KD, S], BF16, name="xb")
        self.wq_sb = sb.tile([P, KD, DL], BF16, name="wq_sb")
        self.wk_sb = sb.tile([P, KD, DL], BF16, name="wk_sb")
        self.wv_sb = sb.tile([P, KD, DL], BF16, name="wv_sb")
        self.wfc_sb = sb.tile([P, DLC, D], BF16, name="wfc_sb")
        self.qT = sb.tile([P, DLC, S], BF16, name="qT")
        self.kT = sb.tile([P, DLC, S], BF16, name="kT")
        self.vN = sb.tile([P, NM, DL], BF16, name="vN")
        self.outT = sb.tile([P, DLC, S], BF16, name="outT")
        self.pending = []

        # loads on the sync+scalar HWDGE queues; gpsimd (slow SWDGE) only for
        # wv/wfc which are needed late. wq + x block0 first for the q0 start.
        nc.scalar.dma_start(self.wq_sb[:, ds(0, 2), :], self.wq[:, ds(0, 2), :])
        nc.sync.dma_start(self.xb[:, 0, ds(0, SQ)], self.xT[:, 0, ds(0, SQ)])
        nc.scalar.dma_start(self.wq_sb[:, ds(2, 6), :], self.wq[:, ds(2, 6), :])
        for k in range(1, 4):
            nc.sync.dma_start(self.xb[:, k, ds(0, SQ)], self.xT[:, k, ds(0, SQ)])
        for k in range(4):
            nc.scalar.dma_start(self.xb[:, k + 4, ds(0, SQ)], self.xT[:, k + 4, ds(0, SQ)])
        nc.scalar.dma_start(self.wk_sb[:], self.wk[:, :, :])
        for k in range(4):
            nc.sync.dma_start(self.xb[:, k, ds(SQ, SQ)], self.xT[:, k, ds(SQ, SQ)])
            nc.scalar.dma_start(self.xb[:, k + 4, ds(SQ, SQ)], self.xT[:, k + 4, ds(SQ, SQ)])
        # wv must beat the v-group injections in window (0,0): HWDGE, not the
        # slow gpsimd SWDGE path (a late wv stalls the whole compile-time-
        # ordered PE queue behind the first v group)
        nc.sync.dma_start(self.wv_sb[:], self.wv[:, :, :])
        for k in range(4):
            nc.sync.dma_start(self.xb[:, k, ds(S // 2, S // 2)], self.xT[:, k, ds(S // 2, S // 2)])
            nc.scalar.dma_start(self.xb[:, k + 4, ds(S // 2, S // 2)], self.xT[:, k + 4, ds(S // 2, S // 2)])
        nc.gpsimd.dma_start(self.wfc_sb[:], self.wfc[:, :, :])

        K = lambda c, nb: self.kq_group(self.wk_sb, self.kT, c, nb)
        Q = lambda c, nb: self.kq_group(self.wq_sb, self.qT, c, nb)

        def fK(c, nb):
            return lambda: K(c, nb)
        def fQ(c, nb):
            return lambda: Q(c, nb)
        def fV(m):
            return lambda: self.v_group(m)
        def fFC(sc, eb):
            return lambda: self.fc_group(sc, eb)

        # upfront: q block 0 + first half of k(c0) - only what scores(0,0)
        # m<8 needs, so the window starts as soon as x blocks 0-1 land
        mt0 = self.load_mask(0)
        Q(0, 0)
        Q(1, 0)
        K(0, 0)
        K(0, 1)

        # qb0 hp0: inject the v-projection (av lag 4 leaves v(m) two its of
        # lead time), the rest of k(c0), and the first k(c1) groups
        inj00 = {j: [fV(j)] for j in range(NM)}
        inj00[6].insert(0, fK(0, 2))
        inj00[10].insert(0, fK(0, 3))
        inj00[16] = [fK(1, 0)]
        inj00[17] = [fK(1, 1)]
        self.window(0, 0, mt0, inj=inj00, batch2=False)
        # qb0 hp1: remaining k(c1) + q block 1
        self.window(0, 1, mt0, inj={
            1: [fK(1, 2)], 5: [fK(1, 3)], 9: [fQ(0, 1)], 13: [fQ(1, 1)],
        })

        for qb in range(1, NQB):
            mt = self.load_mask(qb)
            for hp in range(DLC):
                a = (qb - 1) * 4 + 2 * hp  # fc seq chunks of the previous qb
                self.pending += [fFC(a, 0), fFC(a, 1), fFC(a + 1, 0), fFC(a + 1, 1)]
                if qb < NQB - 1:
                    self.pending.append(fQ(hp, qb + 1))
                self.window(qb, hp, mt)

        # tail: fc for the last qb
        for sc in range(12, 16):
            for eb in range(2):
                self.fc_group(sc, eb, tail=True)
        self.inject(len(self.pending))

        stack.close()


# ---- host wrapper ---------------------------------------------------------

N_HEAD = 16
_nc_cache = {}


def get_nc(with_mask: bool):
    if with_mask not in _nc_cache:
        _nc_cache[with_mask] = build_nc(with_mask)
    return _nc_cache[with_mask]


def make_in_maps(x, mask, Wq, Wk, Wv, Wfc, with_mask):
    scale = np.float32(1.0 / np.sqrt(D // N_HEAD))
    bf = ml_dtypes.bfloat16
    in_maps = []
    for c in range(8):
        b, hg = divmod(c, 4)
        gs = slice(DL * hg, DL * hg + DL)
        def prearrange(wT, cdim):  # [cdim*128, F] -> [128, cdim, F]
            F = wT.shape[1]
            return np.ascontiguousarray(
                wT.reshape(cdim, P, F).transpose(1, 0, 2)
            ).astype(bf)

        m = {
            "xT": prearrange(x[b].T, KD),
            "wq": prearrange((Wq[gs, :] * scale).T, KD),
            "wk": prearrange(Wk[gs, :].T, KD),
            "wv": prearrange(Wv[gs, :].T, KD),
            "wfc": prearrange(Wfc[:, gs].T, DLC),
        }
        if with_mask:
            m["maskT"] = np.ascontiguousarray(
                np.broadcast_to(mask, (1, 1, S, S))[0, 0].T.astype(np.float32)
            )
        in_maps.append(m)
    return in_maps


def kernel(x, mask, Wq, Wk, Wv, Wfc, bfc):
    """Full-input entry: shards across 8 trn2 cores, returns the full output."""
    from concourse.bass_utils import run_bass_kernel_spmd

    x = np.asarray(x, dtype=np.float32)
    mask = np.asarray(mask, dtype=np.float32)
    Wq = np.asarray(Wq, dtype=np.float32)
    Wk = np.asarray(Wk, dtype=np.float32)
    Wv = np.asarray(Wv, dtype=np.float32)
    Wfc = np.asarray(Wfc, dtype=np.float32)
    bfc = np.asarray(bfc, dtype=np.float32)

    B = x.shape[0]
    with_mask = bool(np.any(mask))
    nc = get_nc(with_mask)
    in_maps = make_in_maps(x, mask, Wq, Wk, Wv, Wfc, with_mask)

    res = run_bass_kernel_spmd(nc, in_maps, core_ids=list(range(8)))
    parts = np.stack([np.asarray(r["y"]) for r in res.results])  # [8, S, D] bf16
    out = parts.astype(np.float64).reshape(B, 4, S, D).sum(axis=1)
    out += bfc.astype(np.float64)
    return out.astype(np.float32)


# revision 39
# speedup vs baseline: 1.1055x; 1.0153x over previous
"""Bass/Tile kernel for nn_MultiHeadAttention (B=2, S=2048, D=1024, H=16) on 8 trn2 cores.

Sharding: core c -> (b = c//4, head-group hg = c%4). Each core computes 4 heads'
q/k/v projections, relu-attention, and a partial FC (256 of 1024 contraction rows).
Host pre-casts to bf16, pre-transposes x / weight slices, and sums the 4
partials per batch + bias.

v10 design (baseline v3 structure + absorbed projections):
  - attention windows keep the v3 per-m interleave (scores pair, av pair lag 2,
    one filler slot every other m) - measured fastest: window pace is set by
    the PSUM drain (two single-bank relus per m, one on DVE one on ACT,
    ~680ns/m) and the fine interleave lets the Tile scheduler fill PE gaps
  - scores pairs 2x row-tiled (tile_position (0,0)/(64,0)); av pairs 2x
    col-tiled ((0,0)/(0,64)) - concurrent PE tiles
  - v-projection and the hp1 half of the k-projection are injected into the
    qb0 windows' drain slack; q-projections for qb+1 and fc for qb-1 are
    injected into later windows (v3 ran k/v as standalone PE phases with
    DVE/ACT idle)
  - y output bf16 (halves the 8MB output DMA), staged via [P,1024] tiles
  - weights/x ride the sync+scalar HWDGE queues (gpsimd DMA = slow SWDGE),
    wv/wfc needed late ride gpsimd
"""
import numpy as np
import ml_dtypes

import concourse.bass as bass
import concourse.mybir as mybir
import concourse.tile as tile

F32 = mybir.dt.float32
BF16 = mybir.dt.bfloat16
ts, ds = bass.ts, bass.ds

S = 2048
D = 1024
DL = 256      # per-core q/k/v dim (4 heads x 64)
P = 128
KD = D // P   # 8 k-chunks for projections
SQ = 512      # q-block (matmul N)
NQB = S // SQ # 4
NM = S // P   # 16 kpos chunks
DLC = DL // P # 2


def split_excess_waits(nc, max_embed: int = 1):
    """walrus core_v3 codegen accepts at most one sync-wait per instruction;
    move extra waits onto standalone event-sem instructions inserted before."""
    n_split = 0
    counter = 0
    for f in nc.m.functions:
        for blk in f.blocks:
            insts = blk.instructions
            if not any(
                ins.sync_info is not None and len(ins.sync_info.on_wait) > max_embed
                for ins in insts
            ):
                continue
            newl = []
            for ins in insts:
                si = ins.sync_info
                if si is not None and len(si.on_wait) > max_embed:
                    waits = list(si.on_wait)
                    extra, keep = waits[:-max_embed], waits[-max_embed:]
                    for w in extra:
                        counter += 1
                        es = mybir.InstEventSemaphore(name=f"waitsplit_{counter}")
                        es.engine = ins.engine
                        es.sync_info = mybir.SyncInfo(on_wait=[w], on_update=[])
                        newl.append(es)
                        n_split += 1
                    si.on_wait = keep
                newl.append(ins)
            blk.instructions = newl
    return n_split


def build_nc(with_mask: bool):
    nc = bass.Bass()
    # pre-arranged on host: x[p, c, s] = x.T[128c+p, s]; w[p, c, f] = w.T[128c+p, f]
    xT = nc.dram_tensor("xT", [P, KD, S], BF16, kind="ExternalInput")
    wq = nc.dram_tensor("wq", [P, KD, DL], BF16, kind="ExternalInput")
    wk = nc.dram_tensor("wk", [P, KD, DL], BF16, kind="ExternalInput")
    wv = nc.dram_tensor("wv", [P, KD, DL], BF16, kind="ExternalInput")
    wfc = nc.dram_tensor("wfc", [P, DLC, D], BF16, kind="ExternalInput")
    maskT = nc.dram_tensor("maskT", [S, S], F32, kind="ExternalInput") if with_mask else None
    y = nc.dram_tensor("y", [S, D], BF16, kind="ExternalOutput")

    with tile.TileContext(nc) as tc:
        _Emitter(tc, xT, wq, wk, wv, wfc, maskT, y).run()
    split_excess_waits(nc)
    return nc


class _Emitter:
    def __init__(self, tc, xT, wq, wk, wv, wfc, maskT, y):
        self.tc = tc
        self.nc = tc.nc
        self.xT, self.wq, self.wk, self.wv, self.wfc = xT, wq, wk, wv, wfc
        self.maskT, self.y = maskT, y
        self.cp = 0
        self.rl = 0

    # -- engine alternation helpers ----------------------------------------
    def copyback(self, out_ap, in_ap):
        if self.cp % 2 == 0:
            self.nc.vector.tensor_copy(out_ap, in_ap)
        else:
            self.nc.scalar.copy(out_ap, in_ap)
        self.cp += 1

    def relu(self, out_ap, in_ap):
        if self.rl % 2 == 0:
            self.nc.vector.tensor_scalar_max(out_ap, in_ap, 0.0)
        else:
            self.nc.scalar.activation(out_ap, in_ap, mybir.ActivationFunctionType.Relu)
        self.rl += 1

    # -- emission pieces ----------------------------------------------------
    def kq_group(self, wsb, dstT, c, nb):
        """one projection psum group: dstT[:, c, nb*SQ:...] via 8 k-chunk matmuls"""
        nc = self.nc
        pt = self.ps.tile([P, SQ], F32, tag="pp", name=f"pj_{dstT.name}_{c}_{nb}")
        for k in range(KD):
            nc.tensor.matmul(
                pt[:], wsb[:, k, ts(c, P)], self.xb[:, k, ds(nb * SQ, SQ)],
                start=(k == 0), stop=(k == KD - 1),
            )
        self.copyback(dstT[:, c, ds(nb * SQ, SQ)], pt[:])

    def v_group(self, m):
        nc = self.nc
        pt = self.ps.tile([P, DL], F32, tag="pp", name=f"v_{m}")
        for k in range(KD):
            nc.tensor.matmul(
                pt[:], self.xb[:, k, ts(m, P)], self.wv_sb[:, k, :],
                start=(k == 0), stop=(k == KD - 1),
            )
        self.copyback(self.vN[:, m, :], pt[:])

    def scores_single(self, qb, hp, m, attn_t, mtile):
        """one scores pair: 2 row-tiled concurrent matmuls, two single-bank
        relus alternating DVE/ACT (measured best: paired [P,2,SQ] relus double
        the drain latency per pair and stall the 2-pair PSUM rotation)"""
        nc = self.nc
        pts = []
        for h in range(2):
            pt = self.ps.tile([P, SQ], F32, tag="pp", name=f"sc_{qb}_{hp}_{m}_{h}")
            nc.tensor.matmul(
                pt[:],
                self.kT[ds(64 * h, 64), hp, ts(m, P)],
                self.qT[ds(64 * h, 64), hp, ds(qb * SQ, SQ)],
                start=True, stop=True,
                tile_position=(64 * h, 0),
            )
            pts.append(pt)
        for h in range(2):
            pt = pts[h]
            if mtile is not None:
                nc.vector.tensor_tensor(
                    pt[:], pt[:], mtile[:, m, :], mybir.AluOpType.add
                )
            self.relu(attn_t[:, m, h, :], pt[:])

    def av(self, qb, hp, m, attn_t, po):
        nc = self.nc
        for h in range(2):
            nc.tensor.matmul(
                po[ds(64 * h, 64), :],
                self.vN[:, m, ds(128 * hp + 64 * h, 64)],
                attn_t[:, m, h, :],
                start=(m == 0), stop=(m == NM - 1),
                tile_position=(0, 64 * h),
            )

    def fc_group(self, sc, eb, tail=False):
        """single-bank fc group: y rows [sc] cols [eb-half]; staged to bf16.
        tail groups alternate into the pp pool (free after the last scores)
        so the final 8 groups don't serialize on 2 fc banks."""
        nc = self.nc
        if tail and (sc + eb) % 2 == 0:
            pt = self.ps.tile([P, SQ], F32, tag="pp", name=f"fc_{sc}_{eb}")
        else:
            pt = self.ps_fc.tile([P, SQ], F32, tag="fc", name=f"fc_{sc}_{eb}")
        for c in range(DLC):
            nc.tensor.matmul(
                pt[:], self.outT[:, c, ts(sc, P)], self.wfc_sb[:, c, ds(eb * SQ, SQ)],
                start=(c == 0), stop=(c == DLC - 1),
            )
        if eb == 0:
            self.cur_yt = self.ystage.tile([P, D], BF16, tag="yt", name=f"yt_{sc}")
        self.copyback(self.cur_yt[:, ds(eb * SQ, SQ)], pt[:])
        if eb == 1:
            nc.sync.dma_start(self.y[ts(sc, P), :], self.cur_yt[:])

    def inject(self, n=1):
        for _ in range(n):
            if self.pending:
                self.pending.pop(0)()

    def load_mask(self, qb):
        if self.maskT is None:
            return None
        nc = self.nc
        mtile = self.mstg.tile([P, NM, SQ], F32, tag="mask", name=f"mask_{qb}")
        for m in range(NM):
            nc.gpsimd.dma_start(
                mtile[:, m, :],
                self.maskT[:, :].rearrange("(m p) q -> p m q", p=P)[:, m, ds(qb * SQ, SQ)],
            )
        return mtile

    def window(self, qb, hp, mt, inj=None, batch2=True):
        """attention window: scores pairs, av pairs (lag 4), injections.
        inj: dict m -> list of group fns (k/v/q/fc) to emit at that iteration;
        self.pending drains one slot every other m when no explicit inj."""
        at = self.attn_pool.tile([P, NM, 2, SQ], BF16, tag="attn", name=f"attn_{qb}_{hp}")
        po = self.ps_av.tile([P, SQ], F32, tag="av", name=f"av_{qb}_{hp}")
        inj = inj or {}
        if batch2:
            # 2-pair scores / 2-pair av (lag 4) batches: halves the row<->col
            # mode transitions vs per-m alternation (~110ns per transition)
            for m in range(NM + 4):
                for f in inj.get(m, ()):
                    f()
                if m < NM and m % 2 == 0:
                    self.scores_single(qb, hp, m, at, mt)
                    self.scores_single(qb, hp, m + 1, at, mt)
                if m >= 4 and m % 2 == 0:
                    self.av(qb, hp, m - 4, at, po)
                    self.av(qb, hp, m - 3, at, po)
                if m % 2 == 1 and m not in inj:
                    self.inject(1)
        else:
            # per-m interleave with av lag 4: used for the v-heavy qb0 hp0
            # window where an injected group sits between v(m) and av(m)
            for m in range(NM + 4):
                for f in inj.get(m, ()):
                    f()
                if m < NM:
                    self.scores_single(qb, hp, m, at, mt)
                if m >= 4:
                    self.av(qb, hp, m - 4, at, po)
        self.copyback(self.outT[:, hp, ds(qb * SQ, SQ)], po[:])

    # -- main ---------------------------------------------------------------
    def run(self):
        from contextlib import ExitStack

        tc, nc = self.tc, self.nc
        stack = ExitStack()
        sb = stack.enter_context(tc.tile_pool(name="sb", bufs=1))
        # PSUM budget (8 banks): pp 4, fc 2, av 2
        self.ps = stack.enter_context(tc.tile_pool(name="ps", bufs=4, space="PSUM"))
        self.ps_fc = stack.enter_context(tc.tile_pool(name="ps_fc", bufs=2, space="PSUM"))
        self.ps_av = stack.enter_context(tc.tile_pool(name="ps_av", bufs=2, space="PSUM"))
        self.attn_pool = stack.enter_context(tc.tile_pool(name="attn", bufs=2))
        self.mstg = stack.enter_context(tc.tile_pool(name="mstg", bufs=2))
        self.ystage = stack.enter_context(tc.tile_pool(name="ystage", bufs=4))

        self.xb = sb.tile([P, KD, S], BF16, name="xb")
        self.wq_sb = sb.tile([P, KD, DL], BF16, name="wq_sb")
        self.wk_sb = sb.tile([P, KD, DL], BF16, name="wk_sb")
        self.wv_sb = sb.tile([P, KD, DL], BF16, name="wv_sb")
        self.wfc_sb = sb.tile([P, DLC, D], BF16, name="wfc_sb")
        self.qT = sb.tile([P, DLC, S], BF16, name="qT")
        self.kT = sb.tile([P, DLC, S], BF16, name="kT")
        self.vN = sb.tile([P, NM, DL], BF16, name="vN")
        self.outT = sb.tile([P, DLC, S], BF16, name="outT")
        self.pending = []

        # loads on the sync+scalar HWDGE queues; gpsimd (slow SWDGE) only for
        # wv/wfc which are needed late. wq + x block0 first for the q0 start.
        nc.scalar.dma_start(self.wq_sb[:, ds(0, 2), :], self.wq[:, ds(0, 2), :])
        nc.sync.dma_start(self.xb[:, 0, ds(0, SQ)], self.xT[:, 0, ds(0, SQ)])
        nc.scalar.dma_start(self.wq_sb[:, ds(2, 6), :], self.wq[:, ds(2, 6), :])
        for k in range(1, 4):
            nc.sync.dma_start(self.xb[:, k, ds(0, SQ)], self.xT[:, k, ds(0, SQ)])
        for k in range(4):
            nc.scalar.dma_start(self.xb[:, k + 4, ds(0, SQ)], self.xT[:, k + 4, ds(0, SQ)])
        # wv must beat the v-group injections in window (0,0): HWDGE, not the
        # slow gpsimd SWDGE path (a late wv stalls the whole compile-time-
        # ordered PE queue behind the first v group)
        nc.sync.dma_start(self.wv_sb[:], self.wv[:, :, :])
        nc.scalar.dma_start(self.wk_sb[:], self.wk[:, :, :])
        for k in range(4):
            nc.sync.dma_start(self.xb[:, k, ds(SQ, SQ)], self.xT[:, k, ds(SQ, SQ)])
            nc.scalar.dma_start(self.xb[:, k + 4, ds(SQ, SQ)], self.xT[:, k + 4, ds(SQ, SQ)])
        for k in range(4):
            nc.sync.dma_start(self.xb[:, k, ds(S // 2, S // 2)], self.xT[:, k, ds(S // 2, S // 2)])
            nc.scalar.dma_start(self.xb[:, k + 4, ds(S // 2, S // 2)], self.xT[:, k + 4, ds(S // 2, S // 2)])
        nc.gpsimd.dma_start(self.wfc_sb[:], self.wfc[:, :, :])

        K = lambda c, nb: self.kq_group(self.wk_sb, self.kT, c, nb)
        Q = lambda c, nb: self.kq_group(self.wq_sb, self.qT, c, nb)

        def fK(c, nb):
            return lambda: K(c, nb)
        def fQ(c, nb):
            return lambda: Q(c, nb)
        def fV(m):
            return lambda: self.v_group(m)
        def fFC(sc, eb):
            return lambda: self.fc_group(sc, eb)

        # upfront: q block 0 + first half of k(c0) - only what scores(0,0)
        # m<8 needs, so the window starts as soon as x blocks 0-1 land
        mt0 = self.load_mask(0)
        Q(0, 0)
        Q(1, 0)
        K(0, 0)

        # qb0 hp0: inject the v-projection (av lag 4 leaves v(m) two its of
        # lead time), the rest of k(c0), and the first k(c1) groups
        inj00 = {j: [fV(j)] for j in range(NM)}
        inj00[1].insert(0, fK(0, 1))
        inj00[6].insert(0, fK(0, 2))
        inj00[10].insert(0, fK(0, 3))
        inj00[16] = [fK(1, 0)]
        inj00[17] = [fK(1, 1)]
        self.window(0, 0, mt0, inj=inj00, batch2=False)
        # qb0 hp1: remaining k(c1) + q block 1
        self.window(0, 1, mt0, inj={
            1: [fK(1, 2)], 5: [fK(1, 3)], 9: [fQ(0, 1)], 13: [fQ(1, 1)],
        })

        for qb in range(1, NQB):
            mt = self.load_mask(qb)
            for hp in range(DLC):
                a = (qb - 1) * 4 + 2 * hp  # fc seq chunks of the previous qb
                self.pending += [fFC(a, 0), fFC(a, 1), fFC(a + 1, 0), fFC(a + 1, 1)]
                if qb < NQB - 1:
                    self.pending.append(fQ(hp, qb + 1))
                self.window(qb, hp, mt)

        # tail: fc for the last qb
        for sc in range(12, 16):
            for eb in range(2):
                self.fc_group(sc, eb, tail=True)
        self.inject(len(self.pending))

        stack.close()


# ---- host wrapper ---------------------------------------------------------

N_HEAD = 16
_nc_cache = {}


def get_nc(with_mask: bool):
    if with_mask not in _nc_cache:
        _nc_cache[with_mask] = build_nc(with_mask)
    return _nc_cache[with_mask]


def make_in_maps(x, mask, Wq, Wk, Wv, Wfc, with_mask):
    scale = np.float32(1.0 / np.sqrt(D // N_HEAD))
    bf = ml_dtypes.bfloat16
    in_maps = []
    for c in range(8):
        b, hg = divmod(c, 4)
        gs = slice(DL * hg, DL * hg + DL)
        def prearrange(wT, cdim):  # [cdim*128, F] -> [128, cdim, F]
            F = wT.shape[1]
            return np.ascontiguousarray(
                wT.reshape(cdim, P, F).transpose(1, 0, 2)
            ).astype(bf)

        m = {
            "xT": prearrange(x[b].T, KD),
            "wq": prearrange((Wq[gs, :] * scale).T, KD),
            "wk": prearrange(Wk[gs, :].T, KD),
            "wv": prearrange(Wv[gs, :].T, KD),
            "wfc": prearrange(Wfc[:, gs].T, DLC),
        }
        if with_mask:
            m["maskT"] = np.ascontiguousarray(
                np.broadcast_to(mask, (1, 1, S, S))[0, 0].T.astype(np.float32)
            )
        in_maps.append(m)
    return in_maps


def kernel(x, mask, Wq, Wk, Wv, Wfc, bfc):
    """Full-input entry: shards across 8 trn2 cores, returns the full output."""
    from concourse.bass_utils import run_bass_kernel_spmd

    x = np.asarray(x, dtype=np.float32)
    mask = np.asarray(mask, dtype=np.float32)
    Wq = np.asarray(Wq, dtype=np.float32)
    Wk = np.asarray(Wk, dtype=np.float32)
    Wv = np.asarray(Wv, dtype=np.float32)
    Wfc = np.asarray(Wfc, dtype=np.float32)
    bfc = np.asarray(bfc, dtype=np.float32)

    B = x.shape[0]
    with_mask = bool(np.any(mask))
    nc = get_nc(with_mask)
    in_maps = make_in_maps(x, mask, Wq, Wk, Wv, Wfc, with_mask)

    res = run_bass_kernel_spmd(nc, in_maps, core_ids=list(range(8)))
    parts = np.stack([np.asarray(r["y"]) for r in res.results])  # [8, S, D] bf16
    out = parts.astype(np.float64).reshape(B, 4, S, D).sum(axis=1)
    out += bfc.astype(np.float64)
    return out.astype(np.float32)
